# revision 26
# baseline (speedup 1.0000x reference)
"""BitnetMLP on 8 TRN2 NeuronCores — Megatron tensor-parallel over the
intermediate dim I, exact integer matmuls on the TensorEngine.

v2: dense-PE scheduling. Key changes vs v1:
  - qxT / qhT hold PURE int8 values in bf16; the full per-token dequant
    scale (mc = mx/127) is applied post-matmul (r_tile), so x-quant is
    2 vector ops per tile instead of 4 (no pow2/residual split).
  - abs-max chains use the fused abs_max ALU op.
  - clamps after MAGIC-rounding dropped (Newton-iterated reciprocals give
    |xs| <= 127*(1+1e-7) which rounds to <= 127).
  - h^2 runs on the Scalar engine (AF.Square), weight-stat reduces on
    GpSimd, output casts on GpSimd: the Vector engine only does work that
    must be on it.
  - emission is interleaved per-ic-step so each engine's in-order stream
    always has ready work: PE never starves -> stays at the 2.4 GHz
    p-state (idle gaps drop it to 1.2 GHz).
  - per-weight AllReduce of the |w| sums so w_gate quantization (needed
    first) is not blocked on w_up/w_down stats.

Sharding (per core r of 8):
  w_gate/w_up: I-column shard (1024 of 8192) -> h^T shard [1024, T]
  w_down:      I-row shard -> partial y, per-group ReduceScatter(add)
  per-token RMS var / abs-max stats over full I: AllGather + local reduce.
"""
import numpy as np

N_CORES = 8
B, S, H, I = 2, 2048, 2048, 8192
T = B * S                      # 4096 tokens
ISH = I // N_CORES             # 1024  I shard per core
TG = 512                       # tokens per group
NG = T // TG                   # 8 groups
KC = H // 128                  # 16 contract chunks for gate/up
IC = ISH // 128                # 8  contract chunks for down / hT chunks
NH = 2048 // 512               # 4  output col groups for down
NTC = TG // 128                # 4  token tiles per group
NSTEP = IC                     # 8  pipeline steps per group

MAGIC = float(1.5 * 2 ** 23)   # f32 round-to-nearest-even forcing constant
EPS = 1e-5
RMS_EPS = 1e-6

_CACHED = {}


def _build():
    import concourse.bass as bass
    import concourse.bacc as bacc
    import concourse.tile as tile
    import concourse.mybir as mybir
    from concourse import masks
    from contextlib import ExitStack

    dt = mybir.dt
    AO = mybir.AluOpType
    AF = mybir.ActivationFunctionType
    RG = [list(range(N_CORES))]

    nc = bacc.Bacc("TRN2", target_bir_lowering=False, debug=False,
                   num_devices=N_CORES)

    xT_in = nc.dram_tensor("xT", [H, T], dt.float32, kind="ExternalInput")
    wgT_in = nc.dram_tensor("wgT", [H, ISH], dt.float32, kind="ExternalInput")
    wuT_in = nc.dram_tensor("wuT", [H, ISH], dt.float32, kind="ExternalInput")
    wdT_in = nc.dram_tensor("wdT", [ISH, 2048], dt.float32, kind="ExternalInput")
    lnw_in = nc.dram_tensor("lnw", [ISH], dt.float32, kind="ExternalInput")
    y_out = nc.dram_tensor("y_out", [T // N_CORES, 2048], dt.float32,
                           kind="ExternalOutput")

    with tile.TileContext(nc) as tc:
        with ExitStack() as stack:
            ep = stack.enter_context
            constp = ep(tc.tile_pool(name="const", bufs=1))
            wqp = ep(tc.tile_pool(name="wq", bufs=1))
            wstage = ep(tc.tile_pool(name="wstage", bufs=2))
            xstage = ep(tc.tile_pool(name="xstage", bufs=6))
            qxp = ep(tc.tile_pool(name="qx", bufs=2))
            xmp = ep(tc.tile_pool(name="xm", bufs=2))
            hbp = ep(tc.tile_pool(name="hbuf", bufs=3))
            qhp = ep(tc.tile_pool(name="qh", bufs=3))
            sxbp = ep(tc.tile_pool(name="sxb", bufs=2))
            mcbp = ep(tc.tile_pool(name="mcb", bufs=2))
            albp = ep(tc.tile_pool(name="alb", bufs=2))
            yrp = ep(tc.tile_pool(name="yrow", bufs=2))
            smp = ep(tc.tile_pool(name="small", bufs=2))
            rowp = ep(tc.tile_pool(name="rows", bufs=2))
            rowp2 = ep(tc.tile_pool(name="rows2", bufs=1))
            evp = ep(tc.tile_pool(name="evac", bufs=2))
            ps_gu = ep(tc.tile_pool(name="ps_gu", bufs=4, space="PSUM"))
            ps_dn = ep(tc.tile_pool(name="ps_dn", bufs=2, space="PSUM"))
            ps_ss = ep(tc.tile_pool(name="ps_ss", bufs=1, space="PSUM"))
            ps_misc = ep(tc.tile_pool(name="ps_misc", bufs=1, space="PSUM"))
            dram = ep(tc.tile_pool(name="dram", bufs=1, space="DRAM"))
            dram_rs = ep(tc.tile_pool(name="dram_rs", bufs=8, space="DRAM"))

            # ---------- constants ----------
            ident = constp.tile([128, 128], dt.float32)
            masks.make_identity(nc, ident[:])
            ones_col = constp.tile([128, 1], dt.float32)
            nc.vector.memset(ones_col[:], 1.0)
            ones_col_bf = constp.tile([128, 1], dt.bfloat16)
            nc.vector.memset(ones_col_bf[:], 1.0)
            ones_row = constp.tile([1, 128], dt.float32)
            nc.vector.memset(ones_row[:], 1.0)
            lnw_sb = constp.tile([128, IC], dt.float32)    # lnw[128*ic+p] at [p,ic]
            nc.sync.dma_start(lnw_sb[:], lnw_in.rearrange("(c p) -> p c", p=128)[:])
            alnw_sb = constp.tile([128, IC], dt.float32)   # |lnw|
            nc.vector.tensor_scalar(alnw_sb.bitcast(dt.uint32)[:],
                                    lnw_sb.bitcast(dt.uint32)[:],
                                    0x7FFFFFFF, None, AO.bitwise_and)
            lnw2_sb = constp.tile([128, IC], dt.float32)   # lnw^2
            nc.vector.tensor_tensor(lnw2_sb[:], lnw_sb[:], lnw_sb[:], AO.mult)
            # wstats columns: 0..2 = s_g, s_u, s_d (1/mean|w|);
            #                 3..5 = mg, mu, md   (mean|w|)
            wstats = constp.tile([128, 8], dt.float32)

            # ---------- internal DRAM ----------
            y_partial = dram.tile([T, 2048], dt.bfloat16)
            stat_in = dram.tile([NG, 2, TG], dt.float32)
            stat_out = dram.tile([NG // 2, N_CORES, 2, 2, TG], dt.float32)
            wsum_part = dram.tile([3, 8], dt.float32)
            wsum_glob = dram.tile([3, 8], dt.float32)
            row_bounce = dram.tile([NG, 5, TG], dt.float32)  # sx/mc/-/al/cd

            # ---------- weight quant state ----------
            qwg = wqp.tile([128, KC * ISH], dt.float8e4)
            qwu = wqp.tile([128, KC * ISH], dt.float8e4)
            qwd = wqp.tile([128, IC * 2048], dt.float8e4)

            def w_chunks(wi):
                """Uniform [128, 1024] f32 chunk list: (dram_ap, qw_slice)."""
                w_in = (wgT_in, wuT_in, wdT_in)[wi]
                qw = (qwg, qwu, qwd)[wi]
                wcols = (ISH, ISH, 2048)[wi]
                nrow = (KC, KC, IC)[wi]
                out = []
                for r in range(nrow):
                    for cc in range(wcols // 1024):
                        out.append((
                            w_in[r * 128:(r + 1) * 128,
                                 cc * 1024:(cc + 1) * 1024],
                            qw[:, r * wcols + cc * 1024:
                               r * wcols + cc * 1024 + 1024]))
                return out

            def emit_wstats(wi):
                """DMA weight chunks, |.|-sum on gpsimd, PE column-sum, AR."""
                acc = smp.tile([128, 1], dt.float32, tag=f"wacc{wi}")
                for c, (src, _) in enumerate(w_chunks(wi)):
                    st = wstage.tile([128, 1024], dt.float32, tag="wstage")
                    nc.sync.dma_start(st[:], src)
                    red = smp.tile([128, 1], dt.float32, tag=f"wred{wi}")
                    nc.vector.tensor_reduce(red[:], st[:], mybir.AxisListType.X,
                                            AO.add, apply_absolute_value=True)
                    if c == 0:
                        nc.vector.tensor_copy(acc[:], red[:])
                    else:
                        nc.vector.tensor_tensor(acc[:], acc[:], red[:], AO.add)
                ws_ps = ps_misc.tile([128, 512], dt.float32, tag="misc_ps")
                nc.tensor.matmul(ws_ps[0:1, 0:1], ones_col[:], acc[:],
                                 start=True, stop=True)
                wrow = rowp.tile([1, 8], dt.float32, tag=f"wrow{wi}")
                nc.vector.memset(wrow[:], 0.0)
                nc.scalar.copy(wrow[:, 0:1], ws_ps[0:1, 0:1])
                nc.sync.dma_start(wsum_part[wi].rearrange("(o f) -> o f", o=1)[:],
                                  wrow[:])
                nc.gpsimd.collective_compute(
                    "AllReduce", AO.add, replica_groups=RG,
                    ins=[wsum_part[wi].opt()], outs=[wsum_glob[wi].opt()])

            def emit_wscale(wi):
                """mean = sum/(I*H); s = 1/mean (Newton); broadcast to wstats."""
                wg_row = rowp.tile([1, 2], dt.float32, tag=f"wsg{wi}")
                nc.sync.dma_start(wg_row[:, 0:1],
                                  wsum_glob[wi, 0:1].rearrange("(o f) -> o f", o=1)[:])
                mean = rowp.tile([1, 1], dt.float32, tag=f"wmean{wi}")
                nc.vector.tensor_scalar(mean[:], wg_row[:, 0:1],
                                        float(1.0 / (I * H)), EPS, AO.mult, AO.max)
                r0 = rowp.tile([1, 1], dt.float32, tag=f"wr0{wi}")
                nc.vector.reciprocal(r0[:], mean[:])
                nt = rowp.tile([1, 1], dt.float32, tag=f"wnt{wi}")
                nc.vector.tensor_tensor(nt[:], mean[:], r0[:], AO.mult)
                nc.vector.tensor_scalar(nt[:], nt[:], -1.0, 2.0, AO.mult, AO.add)
                sm = rowp.tile([1, 2], dt.float32, tag=f"wsm{wi}")
                nc.vector.tensor_tensor(sm[:, 0:1], r0[:], nt[:], AO.mult)
                nc.vector.tensor_copy(sm[:, 1:2], mean[:])
                bc_ps = ps_misc.tile([128, 512], dt.float32, tag="misc_ps")
                nc.tensor.matmul(bc_ps[:, 0:2], ones_row[:], sm[:],
                                 start=True, stop=True)
                nc.vector.tensor_copy(wstats[:, wi:wi + 1], bc_ps[:, 0:1])
                nc.vector.tensor_copy(wstats[:, 3 + wi:4 + wi], bc_ps[:, 1:2])

            def emit_wquant(wi):
                """Ternary fake-quant to fp8 on the vector engine."""
                for c, (src, dst) in enumerate(w_chunks(wi)):
                    nc_e = nc.vector
                    st = wstage.tile([128, 1024], dt.float32, tag="wstage")
                    nc.sync.dma_start(st[:], src)
                    nc_e.tensor_scalar(st[:], st[:], wstats[:, wi:wi + 1],
                                       MAGIC, AO.mult, AO.add)
                    nc_e.tensor_scalar(st[:], st[:], -MAGIC, 1.0, AO.add, AO.min)
                    nc_e.tensor_scalar(dst, st[:], -1.0, None, AO.max)

            # ---------- x prepass state ----------
            xmax_slots = {}
            sx_slots = {}
            mc_slots = {}
            qxT_slots = {}
            al_slots = {}
            cd_slots = {}
            hT_slots = {}
            qh_slots = {}
            rs_slots = {}

            def emit_abs2(g, j2):
                """Two x tiles: DMA + abs-max accumulate (vector)."""
                tok0 = g * TG
                if j2 == 0:
                    xmax_slots[g] = xmp.tile([128, TG], dt.float32, tag="xmax", name="xmax")
                xmax = xmax_slots[g]
                for kc in (2 * j2, 2 * j2 + 1):
                    st = xstage.tile([128, TG], dt.float32, tag="xs")
                    nc.sync.dma_start(st[:], xT_in[kc * 128:(kc + 1) * 128,
                                                   tok0:tok0 + TG])
                    if kc == 0:
                        nc.vector.tensor_scalar(xmax[:], st[:], -1.0, None,
                                                AO.mult)
                    else:
                        nc.vector.scalar_tensor_tensor(xmax[:], st[:], -1.0,
                                                       xmax[:], AO.mult, AO.max)
                    nc.vector.tensor_tensor(xmax[:], st[:], xmax[:], AO.max)

            def emit_sx(g):
                """Per-token max -> sx = 127/mx, mc = mx/127; broadcast tiles."""
                xmax = xmax_slots.pop(g)
                mx_nat = smp.tile([128, NTC], dt.float32, tag="mx_nat")
                for c in range(NTC):
                    tr_ps = ps_misc.tile([128, 512], dt.float32, tag="misc_ps")
                    nc.tensor.transpose(tr_ps[:, 0:128],
                                        xmax[:, c * 128:(c + 1) * 128], ident[:])
                    nc.vector.tensor_reduce(mx_nat[:, c:c + 1], tr_ps[:, 0:128],
                                            mybir.AxisListType.X, AO.max)
                nc.vector.tensor_scalar(mx_nat[:], mx_nat[:], EPS, None, AO.max)
                r0 = smp.tile([128, NTC], dt.float32, tag="sx_r0")
                nc.vector.reciprocal(r0[:], mx_nat[:])
                ntr = smp.tile([128, NTC], dt.float32, tag="sx_nt")
                nc.vector.tensor_tensor(ntr[:], mx_nat[:], r0[:], AO.mult)
                nc.vector.tensor_scalar(ntr[:], ntr[:], -1.0, 2.0, AO.mult, AO.add)
                sxmc = smp.tile([128, 2 * NTC], dt.float32, tag="sxmc")
                nc.vector.tensor_tensor(sxmc[:, 0:NTC], r0[:], ntr[:], AO.mult)
                nc.vector.tensor_scalar(sxmc[:, 0:NTC], sxmc[:, 0:NTC],
                                        127.0, None, AO.mult)
                nc.vector.tensor_scalar(sxmc[:, NTC:2 * NTC], mx_nat[:],
                                        float(1.0 / 127.0), None, AO.mult)
                nc.sync.dma_start(
                    row_bounce[g, 0:2].rearrange("s (c p) -> p s c", p=128)[:],
                    sxmc.rearrange("p (s c) -> p s c", c=NTC)[:])
                sx_tile = sxbp.tile([128, TG], dt.float32, tag="sx_tile")
                sx_slots[g] = sx_tile
                nc.sync.dma_start(sx_tile[:], row_bounce[g, 0]
                                  .rearrange("(o f) -> o f", o=1)
                                  .partition_broadcast(128))
                mc_tile = mcbp.tile([128, TG], dt.float32, tag="mc_tile")
                mc_slots[g] = mc_tile
                nc.sync.dma_start(mc_tile[:], row_bounce[g, 1]
                                  .rearrange("(o f) -> o f", o=1)
                                  .partition_broadcast(128))

            def emit_quant2(g, j2):
                """Two x tiles: re-DMA + (mult sx, MAGIC-round -> bf16)."""
                tok0 = g * TG
                if j2 == 0:
                    qxT_slots[g] = qxp.tile([128, KC * TG], dt.bfloat16,
                                            tag="qxT", name="qxT")
                qxT = qxT_slots[g]
                sx_tile = sx_slots[g]
                for kc in (2 * j2, 2 * j2 + 1):
                    st = xstage.tile([128, TG], dt.float32, tag="xs")
                    nc.sync.dma_start(st[:], xT_in[kc * 128:(kc + 1) * 128,
                                                   tok0:tok0 + TG])
                    nc.vector.tensor_tensor(st[:], st[:], sx_tile[:], AO.mult)
                    nc.vector.tensor_scalar(qxT[:, kc * TG:(kc + 1) * TG], st[:],
                                            MAGIC, -MAGIC, AO.add, AO.add)
                if j2 == KC // 2 - 1:
                    sx_slots.pop(g)

            # ---------- phase 1: gate/up matmuls + evac ----------
            p1_state = {}

            def emit_phase1_step(g, ic):
                if ic == 0:
                    hT = hbp.tile([128, IC * TG], dt.bfloat16, tag="hT")
                    hT_slots[g] = hT
                    st1 = {
                        "hT": hT,
                        "maxt": smp.tile([128, TG], dt.float32, tag="maxt", name="maxt"),
                        "ss_ps": ps_ss.tile([1, TG], dt.float32, tag="ss_ps", name="ss_ps"),
                        "h2": [None] * IC,
                    }
                    p1_state[g] = st1
                st1 = p1_state[g]
                qxT = qxT_slots[g]
                mc_tile = mc_slots[g]
                g_ps = ps_gu.tile([128, TG], dt.float32, tag="gu_ps")
                u_ps = ps_gu.tile([128, TG], dt.float32, tag="gu_ps")
                for kc in range(KC):
                    nc.tensor.matmul(
                        g_ps[:],
                        qwg[:, kc * ISH + ic * 128: kc * ISH + (ic + 1) * 128],
                        qxT[:, kc * TG:(kc + 1) * TG],
                        start=(kc == 0), stop=(kc == KC - 1))
                for kc in range(KC):
                    nc.tensor.matmul(
                        u_ps[:],
                        qwu[:, kc * ISH + ic * 128: kc * ISH + (ic + 1) * 128],
                        qxT[:, kc * TG:(kc + 1) * TG],
                        start=(kc == 0), stop=(kc == KC - 1))
                if ic >= 2:
                    nc.tensor.matmul(st1["ss_ps"][:], ones_col_bf[:],
                                     st1["h2"][ic - 2][:],
                                     start=(ic == 2), stop=False)
                # evac: G = g_ps*mc (vector), silu (scalar), h (vector),
                # h^2 (scalar), running |h*lnw| max (vector)
                gv = evp.tile([128, TG], dt.float32, tag="gsv")
                nc.vector.tensor_tensor(gv[:], g_ps[:], mc_tile[:], AO.mult)
                sv = evp.tile([128, TG], dt.float32, tag="gsv")
                nc.scalar.activation(sv[:], gv[:], AF.Silu,
                                     scale=wstats[:, 3:4])
                hslice = st1["hT"][:, ic * TG:(ic + 1) * TG]
                nc.vector.scalar_tensor_tensor(hslice, u_ps[:],
                                               wstats[:, 4:5], sv[:],
                                               AO.mult, AO.mult)
                h2 = evp.tile([128, TG], dt.bfloat16, tag="h2")
                nc.vector.tensor_tensor(h2[:], hslice, hslice, AO.mult)
                st1["h2"][ic] = h2
                if ic == 0:
                    nc.vector.tensor_scalar(st1["maxt"][:], h2[:],
                                            lnw2_sb[:, 0:1], None, AO.mult)
                else:
                    nc.vector.scalar_tensor_tensor(st1["maxt"][:], h2[:],
                                                   lnw2_sb[:, ic:ic + 1],
                                                   st1["maxt"][:],
                                                   AO.mult, AO.max)

            def emit_phase1_end(g):
                st1 = p1_state.pop(g)
                mc_slots.pop(g)
                qxT_slots.pop(g)
                nc.tensor.matmul(st1["ss_ps"][:], ones_col_bf[:],
                                 st1["h2"][IC - 2][:], start=False, stop=False)
                nc.tensor.matmul(st1["ss_ps"][:], ones_col_bf[:],
                                 st1["h2"][IC - 1][:], start=False, stop=True)
                pm_nat = smp.tile([128, NTC], dt.float32, tag="pm_nat")
                for c in range(NTC):
                    tr_ps = ps_misc.tile([128, 512], dt.float32, tag="misc_ps")
                    nc.tensor.transpose(tr_ps[:, 0:128],
                                        st1["maxt"][:, c * 128:(c + 1) * 128],
                                        ident[:])
                    nc.vector.tensor_reduce(pm_nat[:, c:c + 1], tr_ps[:, 0:128],
                                            mybir.AxisListType.X, AO.max)
                ss_row = rowp.tile([1, TG], dt.float32, tag="grow")
                nc.vector.tensor_copy(ss_row[:], st1["ss_ps"][:])
                nc.gpsimd.dma_start(stat_in[g, 0].rearrange("(o f) -> o f", o=1)[:],
                                    ss_row[:])
                nc.gpsimd.dma_start(stat_in[g, 1].rearrange("(c p) -> p c", p=128)[:],
                                    pm_nat[:])
                if g % 2 == 1:
                    nc.gpsimd.collective_compute(
                        "AllGather", AO.bypass, replica_groups=RG,
                        ins=[stat_in[g - 1:g + 1].opt()],
                        outs=[stat_out[g // 2].opt()])

            # ---------- phase 2a: global RMS stats + scales ----------
            def emit_phase2a(g):
                J = TG // 32
                stat32 = smp.tile([32, TG], dt.float32, tag="stat32")
                nc.vector.memset(stat32[:], 0.0)
                nc.gpsimd.dma_start(stat32[0:2 * N_CORES, :],
                                    stat_out[g // 2][:, g % 2, :, :])
                st32 = smp.tile([32, TG], dt.float32, tag="st32")
                nc.vector.transpose(st32[:], stat32[:])
                stv = st32.rearrange("p (j h a two) -> p j h two a",
                                     h=2, two=2, a=N_CORES)
                ssg = smp.tile([32, J], dt.float32, tag="ssg")
                nc.vector.tensor_reduce(ssg[:], stv[:, :, 0:1, 0:1, :],
                                        mybir.AxisListType.X, AO.add)
                pmg = smp.tile([32, J], dt.float32, tag="pmg")
                nc.vector.tensor_reduce(pmg[:], stv[:, :, 0:1, 1:2, :],
                                        mybir.AxisListType.X, AO.max)
                nc.scalar.sqrt(pmg[:], pmg[:])   # stat carried pm^2
                # correct tracked h = h_true/mc by the per-token mc
                mc32 = smp.tile([32, J], dt.float32, tag="mc32")
                nc.sync.dma_start(mc32[:], row_bounce[g, 1]
                                  .rearrange("(j q) -> q j", q=32)[:])
                nc.vector.tensor_tensor(pmg[:], pmg[:], mc32[:], AO.mult)
                rr2 = smp.tile([32, J], dt.float32, tag="rr2")
                nc.vector.tensor_tensor(rr2[:], mc32[:], mc32[:], AO.mult)
                nc.vector.tensor_tensor(ssg[:], ssg[:], rr2[:], AO.mult)
                vr = smp.tile([32, J], dt.float32, tag="vr")
                nc.vector.tensor_scalar(vr[:], ssg[:], float(1.0 / I), RMS_EPS,
                                        AO.mult, AO.add)
                sq = smp.tile([32, J], dt.float32, tag="sq")
                nc.scalar.sqrt(sq[:], vr[:])
                rr = smp.tile([32, J], dt.float32, tag="rr")
                nc.vector.reciprocal(rr[:], sq[:])
                ntn = smp.tile([32, J], dt.float32, tag="ntn")
                nc.vector.tensor_tensor(ntn[:], sq[:], rr[:], AO.mult)
                nc.vector.tensor_scalar(ntn[:], ntn[:], -1.0, 2.0, AO.mult, AO.add)
                nc.vector.tensor_tensor(rr[:], rr[:], ntn[:], AO.mult)
                rmc = smp.tile([32, J], dt.float32, tag="rmc")
                nc.vector.tensor_tensor(rmc[:], rr[:], pmg[:], AO.mult)
                nc.vector.tensor_scalar(rmc[:], rmc[:], EPS, None, AO.max)
                cd32 = smp.tile([32, J], dt.float32, tag="cd32")
                nc.vector.tensor_scalar(cd32[:], rmc[:], wstats[0:32, 5:6],
                                        float(1.0 / 127.0), AO.mult, AO.mult)
                nc.sync.dma_start(row_bounce[g, 4]
                                  .rearrange("(j q) -> q j", q=32)[:], cd32[:])
                cd = smp.tile([128, NTC], dt.float32, tag="cd")
                cd_slots[g] = cd
                nc.sync.dma_start(cd[:], row_bounce[g, 4]
                                  .rearrange("(c p) -> p c", p=128)[:])
                ar0 = smp.tile([32, J], dt.float32, tag="ar0")
                nc.vector.reciprocal(ar0[:], rmc[:])
                ntn2 = smp.tile([32, J], dt.float32, tag="ntn2")
                nc.vector.tensor_tensor(ntn2[:], rmc[:], ar0[:], AO.mult)
                nc.vector.tensor_scalar(ntn2[:], ntn2[:], -1.0, 2.0, AO.mult,
                                        AO.add)
                nc.vector.tensor_tensor(ar0[:], ar0[:], ntn2[:], AO.mult)
                al32 = smp.tile([32, J], dt.float32, tag="al32")
                nc.vector.tensor_tensor(al32[:], rr[:], ar0[:], AO.mult)
                nc.vector.tensor_scalar(al32[:], al32[:], 127.0, None, AO.mult)
                nc.vector.tensor_tensor(al32[:], al32[:], mc32[:], AO.mult)
                nc.sync.dma_start(row_bounce[g, 3]
                                  .rearrange("(j q) -> q j", q=32)[:], al32[:])
                al_tile = albp.tile([128, TG], dt.float32, tag="al_tile")
                al_slots[g] = al_tile
                nc.sync.dma_start(al_tile[:], row_bounce[g, 3]
                                  .rearrange("(o f) -> o f", o=1)
                                  .partition_broadcast(128))

            # ---------- phase 2q: quantize h (2 tiles per call) ----------
            def emit_hq2(g, j2):
                if j2 == 0:
                    qh_slots[g] = qhp.tile([128, IC * TG], dt.bfloat16,
                                           tag="qhT", name="qhT")
                qhT = qh_slots[g]
                hT = hT_slots[g]
                al_tile = al_slots[g]
                for ic in (2 * j2, 2 * j2 + 1):
                    tq = evp.tile([128, TG], dt.float32, tag="gsv", name="tq")
                    nc.vector.scalar_tensor_tensor(tq[:],
                                                   hT[:, ic * TG:(ic + 1) * TG],
                                                   lnw_sb[:, ic:ic + 1],
                                                   al_tile[:],
                                                   AO.mult, AO.mult)
                    nc.vector.tensor_scalar(qhT[:, ic * TG:(ic + 1) * TG],
                                            tq[:], MAGIC, -MAGIC,
                                            AO.add, AO.add)
                if j2 == IC // 2 - 1:
                    hT_slots.pop(g)
                    al_slots.pop(g)

            # ---------- down proj: 2 chains of 8 matmuls per step ----------
            dn_state = {}

            def emit_down_step(g, step):
                tok0 = g * TG
                qhT = qh_slots[g]
                cd = cd_slots[g]
                for c in (2 * step, 2 * step + 1):
                    tcx, nh = c // NH, c % NH
                    if nh == 0:
                        dn_state[g] = yrp.tile([128, 2048], dt.bfloat16,
                                               tag="y_row", name="y_row")
                    y_row = dn_state[g]
                    y_ps = ps_dn.tile([128, 512], dt.float32, tag="y_ps")
                    for ic in range(IC):
                        nc.tensor.matmul(
                            y_ps[:],
                            qhT[:, ic * TG + tcx * 128: ic * TG + (tcx + 1) * 128],
                            qwd[:, ic * 2048 + nh * 512: ic * 2048 + (nh + 1) * 512],
                            start=(ic == 0), stop=(ic == IC - 1))
                    nc.scalar.mul(y_row[:, nh * 512:(nh + 1) * 512], y_ps[:],
                                  cd[:, tcx:tcx + 1])
                    if nh == NH - 1:
                        nc.sync.dma_start(
                            y_partial[tok0 + tcx * 128: tok0 + (tcx + 1) * 128, :],
                            y_row[:])
                if step == NSTEP - 1:
                    qh_slots.pop(g)
                    cd_slots.pop(g)
                    if g % 2 == 1:
                        rs_out = dram_rs.tile([2 * TG // N_CORES, 2048],
                                              dt.bfloat16, tag="rs_out",
                                              name="rs_out")
                        rs_slots[g // 2] = rs_out
                        nc.gpsimd.collective_compute(
                            "ReduceScatter", AO.add, replica_groups=RG,
                            ins=[y_partial[tok0 - TG:tok0 + TG, :].opt()],
                            outs=[rs_out.opt()])

            rpb = 2 * TG // N_CORES   # 128 rows per RS batch

            def emit_ycast(b):
                yb = rowp2.tile([rpb, 2048], dt.bfloat16, tag="yb")
                nc.sync.dma_start(yb[:], rs_slots.pop(b)[:])
                for cc in range(4):
                    yf = rowp2.tile([rpb, 512], dt.float32, tag="yf")
                    nc.scalar.copy(yf[:], yb[:, cc * 512:(cc + 1) * 512])
                    nc.sync.dma_start(
                        y_out[b * rpb:(b + 1) * rpb, cc * 512:(cc + 1) * 512],
                        yf[:])

            # ================= emission schedule =================
            # preamble: weight stats (DMA+gpsimd) || x prepass g0/g1 (vector)
            emit_wstats(0)
            for j2 in range(KC // 2):
                emit_abs2(0, j2)
            emit_sx(0)
            for j2 in range(KC // 2):
                emit_quant2(0, j2)
            emit_wscale(0)
            emit_wquant(0)          # gate weights first — PE needs them first
            for j2 in range(KC // 2):
                emit_abs2(1, j2)
            emit_wstats(1)
            emit_sx(1)
            emit_wscale(1)
            emit_wquant(1)
            emit_wstats(2)          # w_down: not needed until down(0) at g=2
            emit_wscale(2)
            emit_wquant(2)

            # main pipeline
            ABS2_CALLS = {0: [0], 1: [1], 2: [2], 3: [3], 4: [4, 5],
                          5: [6, 7], 6: [], 7: []}
            for g in range(NG):
                for ic in range(NSTEP):
                    emit_phase1_step(g, ic)
                    if ic == 0 and g >= 1:
                        emit_phase1_end(g - 1)   # PE work lands after gate(g,0)
                    if g + 2 < NG:
                        for j2 in ABS2_CALLS[ic]:
                            emit_abs2(g + 2, j2)
                    if g + 1 < NG and ic < 4:
                        emit_quant2(g + 1, 2 * ic)
                        emit_quant2(g + 1, 2 * ic + 1)
                    if g >= 2 and ic == 4:
                        emit_phase2a(g - 2)
                    if g >= 2 and 4 <= ic:
                        emit_hq2(g - 2, ic - 4)
                    if g >= 3:
                        emit_down_step(g - 3, ic)
                if g + 2 < NG:
                    emit_sx(g + 2)
                if g == 6:
                    emit_ycast(0)
                if g == 7:
                    emit_ycast(1)

            # tail: drain groups NG-3 .. NG-1
            emit_phase1_end(NG - 1)
            for step in range(NSTEP):
                emit_down_step(NG - 3, step)
                if step == 0:
                    emit_phase2a(NG - 2)
                if step < 4:
                    emit_hq2(NG - 2, step)
            for step in range(NSTEP):
                emit_down_step(NG - 2, step)
                if step == 0:
                    emit_phase2a(NG - 1)
                if step < 4:
                    emit_hq2(NG - 1, step)
            for step in range(NSTEP):
                emit_down_step(NG - 1, step)
            emit_ycast(2)
            emit_ycast(3)

    nc.compile()
    return nc


def _get_nc():
    if "nc" not in _CACHED:
        _CACHED["nc"] = _build()
    return _CACHED["nc"]


def _make_in_maps(x, w_gate, w_up, w_down, ln_weight):
    xf = np.ascontiguousarray(np.asarray(x, dtype=np.float32).reshape(T, H).T)
    wgT = np.asarray(w_gate, dtype=np.float32).T   # [H, I]
    wuT = np.asarray(w_up, dtype=np.float32).T     # [H, I]
    wdT = np.asarray(w_down, dtype=np.float32).T   # [I, H]
    lnw = np.asarray(ln_weight, dtype=np.float32)
    in_maps = []
    for r in range(N_CORES):
        c0 = r * ISH
        in_maps.append({
            "xT": xf,
            "wgT": np.ascontiguousarray(wgT[:, c0:c0 + ISH]),
            "wuT": np.ascontiguousarray(wuT[:, c0:c0 + ISH]),
            "wdT": np.ascontiguousarray(wdT[c0:c0 + ISH, :]),
            "lnw": np.ascontiguousarray(lnw[c0:c0 + ISH]),
        })
    return in_maps


def _assemble(results):
    out = np.empty((T, 2048), dtype=np.float32)
    rows_per_batch = 2 * TG // N_CORES      # 128
    for r in range(N_CORES):
        yr = results[r]["y_out"]
        for b in range(NG // 2):
            t0 = b * 2 * TG + r * rows_per_batch
            out[t0:t0 + rows_per_batch] = \
                yr[b * rows_per_batch:(b + 1) * rows_per_batch]
    return out.reshape(B, S, 2048)


def kernel(x, w_gate, w_up, w_down, ln_weight):
    from concourse import bass_utils

    nc = _get_nc()
    in_maps = _make_in_maps(x, w_gate, w_up, w_down, ln_weight)
    res = bass_utils.run_bass_kernel_spmd(nc, in_maps,
                                          core_ids=list(range(N_CORES)))
    return _assemble(res.results)
